# revision 21
# baseline (speedup 1.0000x reference)
"""Trainium2 Bass kernel for CombinedPriorityLoss (MSE + pairwise ranking hinge).

reference semantics (N = 8192, f32):
    mse   = mean((p - t)^2)
    mask  = t[:,None] > t[None,:]
    hinge = where(mask, relu(p[None,:] - p[:,None] + 0.1), 0)
    loss  = 0.4*mse + 0.6*sum(hinge)/max(sum(mask),1)

Strategy (8 NeuronCores, 1024-row blocks of the 8192^2 pair matrix each):
  PE:  d_ij = p_j - p_i + margin as a rank-2 outer sum, K=6 bf16 matmul
       (each fp32 operand split into 3 bf16 terms -> ~fp32 precision at
       full PE rate; fp32 matmuls run at 1/4 rate and double-issue).
  ACT: ra = sigmoid(-2^100 * t_j + 2^100 * t_i) via the activation's free
       scale/bias -- bit-exact sign (power-of-two scaling is exact), so
       ra in {0, 0.5 (ties), 1}; accum_out sums ra -> pair count.
       Input is t replicated across partitions (t_rep, loaded by DMA).
  DVE: scalar_tensor_tensor h = max(d,0) * ra, accum_out -> hinge sum.
  Host: subtracts the diagonal tie term (0.5*relu(margin) per row),
       count = sigma_sum - 0.5*N, then combines per-core partials.
"""

import numpy as np
from contextlib import ExitStack

import ml_dtypes
import concourse.bacc as bacc
import concourse.tile as tile
from concourse import mybir
from concourse.bass_utils import run_bass_kernel_spmd

N = 8192
N_CORES = 8
ROWS_PER_CORE = N // N_CORES        # 1024
P = 128                             # partitions
I_TILES = ROWS_PER_CORE // P        # 8 row tiles per core
J_CHUNK = 2048                      # 4 PSUM banks per pairwise tile
N_JC = N // J_CHUNK                 # 4 column chunks
MM_FREE = 512                       # max moving free dim per matmul
MARGIN = np.float32(0.1)
BIGT = np.float32(2.0 ** 100)       # exact-pow2 mask scale
RANKING_WEIGHT = 0.6
MSE_WEIGHT = 0.4
KD = 128                            # d-matmul contraction depth: rows 0-5
                                    # carry data, 6-127 are zero padding so
                                    # the operand DMA is full-partition-width
                                    # (K depth is free on the systolic array)

_CACHE = {}


def _split3(x):
    """x (f32) -> three bf16 arrays summing to x within ~2^-25 relative."""
    x = x.astype(np.float32)
    x0 = x.astype(ml_dtypes.bfloat16)
    r = x - x0.astype(np.float32)
    x1 = r.astype(ml_dtypes.bfloat16)
    r2 = r - x1.astype(np.float32)
    x2 = r2.astype(ml_dtypes.bfloat16)
    return x0, x1, x2


def _build():
    f32 = mybir.dt.float32
    bf16 = mybir.dt.bfloat16
    nc = bacc.Bacc("TRN2", target_bir_lowering=False, debug=False,
                   num_devices=N_CORES)
    # packed bf16 matmul operands: [6, 1024 (lhsT) + 8192 (rhs)]
    mm6_ext = nc.declare_dram_parameter("mm6", [KD, ROWS_PER_CORE + N], bf16, isOutput=False)
    # t replicated across partitions, DMA'd in 8 column slices
    trep_ext = nc.declare_dram_parameter("trep", [P, N], f32, isOutput=False)
    # per-core row data: [prow | trow | tbias] = [128, 8+8+8]
    rows_ext = nc.declare_dram_parameter("rows", [P, 3 * I_TILES], f32, isOutput=False)
    out_ext = nc.declare_dram_parameter("out", [1, 3], f32, isOutput=True)

    AF = mybir.ActivationFunctionType
    OP = mybir.AluOpType

    with TileCtx(nc) as (tc, ctx):
        singles = ctx.enter_context(tc.tile_pool(name="singles", bufs=1))
        ra_pool = ctx.enter_context(tc.tile_pool(name="ra", bufs=3))
        tr_pool = ctx.enter_context(tc.tile_pool(name="tr", bufs=2))
        psum = ctx.enter_context(tc.tile_pool(name="psum", bufs=2, space="PSUM"))

        mm6 = singles.tile([KD, ROWS_PER_CORE + N], bf16)
        lhst = mm6[:, 0:ROWS_PER_CORE]
        rhsd = mm6[:, ROWS_PER_CORE:ROWS_PER_CORE + N]
        trep = singles.tile([P, N], f32)
        rows = singles.tile([P, 3 * I_TILES], f32)
        prow = rows[:, 0:I_TILES]
        trow = rows[:, I_TILES:2 * I_TILES]
        tbias = rows[:, 2 * I_TILES:3 * I_TILES]
        hacc = singles.tile([P, I_TILES * N_JC], f32)
        cacc = singles.tile([P, I_TILES * N_JC], f32)
        ones = singles.tile([P, 1], f32)
        stats = singles.tile([P, 3], f32)
        diff = singles.tile([P, I_TILES], f32)
        diff2 = singles.tile([P, I_TILES], f32)
        outs = singles.tile([1, 3], f32)

        # matmul operands on their own ring, staged so piece 0 (lhst + the
        # first j-chunks of rhsd) lands fast; trep slices in consumption
        # order on the other two rings; rows (sigmoid bias) first.
        CUT0 = ROWS_PER_CORE + J_CHUNK
        CUT1 = ROWS_PER_CORE + N // 2
        nc.sync.dma_start(out=mm6[:, 0:CUT0], in_=mm6_ext[:, 0:CUT0])
        nc.sync.dma_start(out=mm6[:, CUT0:CUT1], in_=mm6_ext[:, CUT0:CUT1])
        nc.sync.dma_start(out=mm6[:, CUT1:], in_=mm6_ext[:, CUT1:])
        nc.scalar.dma_start(out=rows[:], in_=rows_ext[:])
        # leading 2048 cols in fine 512-col slices (first sigmoid input),
        # remainder in 1024-col slices, alternating the two spare rings
        bounds = [0, 512, 1024, 1536, 2048, 3072, 4096, 5120, 6144, 7168, N]
        for s in range(len(bounds) - 1):
            eng = nc.gpsimd if s % 2 == 0 else nc.scalar
            eng.dma_start(
                out=trep[:, bounds[s]:bounds[s + 1]],
                in_=trep_ext[:, bounds[s]:bounds[s + 1]])
        nc.vector.memset(ones[:], 1.0)
        # warm the sigmoid table while input DMAs stream
        warm = singles.tile([P, 1], f32)
        nc.scalar.activation(out=warm[:], in_=ones[:], func=AF.Sigmoid)

        for it in range(I_TILES):
            lh = lhst[:, it * P:(it + 1) * P]
            for jc in range(N_JC):
                idx = it * N_JC + jc
                j0 = jc * J_CHUNK
                pd = psum.tile([P, J_CHUNK], f32, tag="pd")
                for h in range(J_CHUNK // MM_FREE):
                    a, b = h * MM_FREE, (h + 1) * MM_FREE
                    nc.tensor.matmul(pd[:, a:b], lh, rhsd[:, j0 + a:j0 + b])
                ra = ra_pool.tile([P, J_CHUNK], f32, tag="ra")
                nc.scalar.activation(
                    out=ra[:], in_=trep[:, j0:j0 + J_CHUNK], func=AF.Sigmoid,
                    scale=-float(BIGT), bias=tbias[:, it:it + 1],
                    accum_out=cacc[:, idx:idx + 1],
                )
                tr = tr_pool.tile([P, J_CHUNK], f32, tag="tr")
                nc.vector.scalar_tensor_tensor(
                    out=tr[:], in0=pd[:], scalar=0.0, in1=ra[:],
                    op0=OP.max, op1=OP.mult,
                    accum_out=hacc[:, idx:idx + 1],
                )

        # per-core MSE partial: sum((p_rows - t_rows)^2)
        nc.vector.tensor_tensor(out=diff[:], in0=prow[:], in1=trow[:], op=OP.subtract)
        nc.vector.scalar_tensor_tensor(
            out=diff2[:], in0=diff[:], scalar=0.0, in1=diff[:],
            op0=OP.add, op1=OP.mult, accum_out=stats[:, 2:3],
        )
        # fold 32 chunk accumulators
        nc.vector.tensor_reduce(stats[:, 0:1], hacc[:], axis=mybir.AxisListType.X, op=OP.add)
        nc.vector.tensor_reduce(stats[:, 1:2], cacc[:], axis=mybir.AxisListType.X, op=OP.add)
        # partition reduction: ones^T @ stats -> [1, 3]
        po = psum.tile([1, 3], f32, tag="pd")
        nc.tensor.matmul(po[:], ones[:], stats[:])
        nc.vector.tensor_copy(out=outs[:], in_=po[:])
        nc.sync.dma_start(out=out_ext[:], in_=outs[:])

    nc.finalize()
    return nc


class TileCtx:
    """TileContext + ExitStack bundle so _build reads linearly."""

    def __init__(self, nc):
        self.nc = nc

    def __enter__(self):
        self._stack = ExitStack()
        tc = self._stack.enter_context(tile.TileContext(self.nc))
        return tc, self._stack

    def __exit__(self, *exc):
        return self._stack.__exit__(*exc)


def _prep_inputs(p, t):
    p = np.ascontiguousarray(p, dtype=np.float32)
    t = np.ascontiguousarray(t, dtype=np.float32)
    ones = np.ones(N, dtype=ml_dtypes.bfloat16)
    pj0, pj1, pj2 = _split3(p)
    rhsd = np.zeros((KD, N), dtype=ml_dtypes.bfloat16)
    rhsd[0] = ones
    rhsd[1] = ones
    rhsd[2] = ones
    rhsd[3] = pj0
    rhsd[4] = pj1
    rhsd[5] = pj2
    trep = np.ascontiguousarray(np.broadcast_to(t, (P, N)))     # [128, N] f32
    in_maps = []
    ones_r = np.ones(ROWS_PER_CORE, dtype=ml_dtypes.bfloat16)
    for k in range(N_CORES):
        sl = slice(k * ROWS_PER_CORE, (k + 1) * ROWS_PER_CORE)
        pr, tr = p[sl], t[sl]
        s0, s1, s2 = _split3(MARGIN - pr)
        lhst = np.zeros((KD, ROWS_PER_CORE), dtype=ml_dtypes.bfloat16)
        lhst[0], lhst[1], lhst[2] = s0, s1, s2
        lhst[3], lhst[4], lhst[5] = ones_r, ones_r, ones_r
        mm6 = np.concatenate([lhst, rhsd], axis=1)
        rows = np.concatenate([
            pr.reshape(P, I_TILES, order="F"),
            tr.reshape(P, I_TILES, order="F"),
            (tr * BIGT).reshape(P, I_TILES, order="F"),
        ], axis=1)
        in_maps.append({
            "mm6": np.ascontiguousarray(mm6),
            "trep": trep,
            "rows": np.ascontiguousarray(rows.astype(np.float32)),
        })
    return p, t, in_maps


def _combine(p, t, results):
    hinge_p2 = sum(float(r["out"][0, 0]) for r in results)
    sig_cnt = sum(float(r["out"][0, 1]) for r in results)
    mse_sq = sum(float(r["out"][0, 2]) for r in results)

    # diagonal tie contribution: 0.5 * relu(d_ii), d_ii ~= margin
    dd = ((MARGIN - p) + p).astype(np.float32)
    diag = 0.5 * float(np.maximum(dd, np.float32(0.0)).sum(dtype=np.float64))

    hinge_sum = hinge_p2 - diag
    count = max(round(sig_cnt - 0.5 * N), 1.0)   # diagonal sigmoid(0) = 0.5
    mse = mse_sq / N
    loss = MSE_WEIGHT * mse + RANKING_WEIGHT * hinge_sum / count
    return np.float32(loss)


def _run(in_maps, trace=False):
    if "nc" not in _CACHE:
        _CACHE["nc"] = _build()
    return run_bass_kernel_spmd(
        _CACHE["nc"], in_maps, core_ids=list(range(N_CORES)), trace=trace,
    )


def kernel(predictions, targets):
    p, t, in_maps = _prep_inputs(predictions, targets)
    br = _run(in_maps)
    return _combine(p, t, br.results)


# revision 28
# speedup vs baseline: 1.0552x; 1.0552x over previous
"""Trainium2 Bass kernel for CombinedPriorityLoss (MSE + pairwise ranking hinge).

reference semantics (N = 8192, f32):
    mse   = mean((p - t)^2)
    mask  = t[:,None] > t[None,:]
    hinge = where(mask, relu(p[None,:] - p[:,None] + 0.1), 0)
    loss  = 0.4*mse + 0.6*sum(hinge)/max(sum(mask),1)

Strategy (8 NeuronCores, 1024-row blocks of the 8192^2 pair matrix each):
  PE:  d_ij = p_j - p_i + margin as a rank-2 outer sum, K=6 bf16 matmul
       (each fp32 operand split into 3 bf16 terms -> ~fp32 precision at
       full PE rate; fp32 matmuls run at 1/4 rate and double-issue).
  ACT: ra = sigmoid(-2^100 * t_j + 2^100 * t_i) via the activation's free
       scale/bias -- bit-exact sign (power-of-two scaling is exact), so
       ra in {0, 0.5 (ties), 1}; accum_out sums ra -> pair count.
       Input is t replicated across partitions (t_rep, loaded by DMA).
  DVE: scalar_tensor_tensor h = max(d,0) * ra, accum_out -> hinge sum.
  Host: subtracts the diagonal tie term (0.5*relu(margin) per row),
       count = sigma_sum - 0.5*N, then combines per-core partials.
"""

import numpy as np
from contextlib import ExitStack

import ml_dtypes
import concourse.bacc as bacc
import concourse.tile as tile
from concourse import mybir
from concourse.bass_utils import run_bass_kernel_spmd

N = 8192
N_CORES = 8
ROWS_PER_CORE = N // N_CORES        # 1024
P = 128                             # partitions
I_TILES = ROWS_PER_CORE // P        # 8 row tiles per core
J_CHUNK = 2048                      # 4 PSUM banks per pairwise tile
N_JC = N // J_CHUNK                 # 4 column chunks
MM_FREE = 512                       # max moving free dim per matmul
MARGIN = np.float32(0.1)
BIGT = np.float32(2.0 ** 100)       # exact-pow2 mask scale
RANKING_WEIGHT = 0.6
MSE_WEIGHT = 0.4
KD = 128                            # d-matmul contraction depth: rows 0-5
                                    # carry data, 6-127 are zero padding so
                                    # the operand DMA is full-partition-width
                                    # (K depth is free on the systolic array)

_CACHE = {}


def _split3(x):
    """x (f32) -> three bf16 arrays summing to x within ~2^-25 relative."""
    x = x.astype(np.float32)
    x0 = x.astype(ml_dtypes.bfloat16)
    r = x - x0.astype(np.float32)
    x1 = r.astype(ml_dtypes.bfloat16)
    r2 = r - x1.astype(np.float32)
    x2 = r2.astype(ml_dtypes.bfloat16)
    return x0, x1, x2


def _build():
    f32 = mybir.dt.float32
    bf16 = mybir.dt.bfloat16
    nc = bacc.Bacc("TRN2", target_bir_lowering=False, debug=False,
                   num_devices=N_CORES)
    # packed bf16 matmul operands [128, 1024 (lhsT) + 8192 (rhs)], shipped
    # as 3 contiguous thirds (piece 0 = lhsT + first rhs chunk)
    MM_PIECE = (ROWS_PER_CORE + N) // 3
    mm6_ext = nc.declare_dram_parameter("mm6", [3, KD, MM_PIECE], bf16, isOutput=False)
    # t replicated across partitions; slice-major DRAM layout so each DMA
    # slice is one contiguous block
    NSL = 8
    SL = N // NSL
    trep_ext = nc.declare_dram_parameter("trep", [NSL, P, SL], f32, isOutput=False)
    # per-core row data: [prow | trow | tbias] = [128, 8+8+8]
    rows_ext = nc.declare_dram_parameter("rows", [P, 3 * I_TILES], f32, isOutput=False)
    out_ext = nc.declare_dram_parameter("out", [1, 3], f32, isOutput=True)

    AF = mybir.ActivationFunctionType
    OP = mybir.AluOpType

    with TileCtx(nc) as (tc, ctx):
        singles = ctx.enter_context(tc.tile_pool(name="singles", bufs=1))
        ra_pool = ctx.enter_context(tc.tile_pool(name="ra", bufs=3))
        tr_pool = ctx.enter_context(tc.tile_pool(name="tr", bufs=2))
        psum = ctx.enter_context(tc.tile_pool(name="psum", bufs=2, space="PSUM"))

        mm6 = singles.tile([KD, ROWS_PER_CORE + N], bf16)
        lhst = mm6[:, 0:ROWS_PER_CORE]
        rhsd = mm6[:, ROWS_PER_CORE:ROWS_PER_CORE + N]
        trep = singles.tile([P, N], f32)
        rows = singles.tile([P, 3 * I_TILES], f32)
        prow = rows[:, 0:I_TILES]
        trow = rows[:, I_TILES:2 * I_TILES]
        tbias = rows[:, 2 * I_TILES:3 * I_TILES]
        hacc = singles.tile([P, I_TILES * N_JC], f32)
        cacc = singles.tile([P, I_TILES * N_JC], f32)
        ones = singles.tile([P, 1], f32)
        stats = singles.tile([P, 3], f32)
        diff = singles.tile([P, I_TILES], f32)
        diff2 = singles.tile([P, I_TILES], f32)
        outs = singles.tile([1, 3], f32)

        # matmul operands on their own ring, staged so piece 0 (lhst + the
        # first rhsd chunk) lands fast; trep slices on the other two rings.
        for pc in range(3):
            nc.sync.dma_start(
                out=mm6[:, pc * MM_PIECE:(pc + 1) * MM_PIECE], in_=mm6_ext[pc])
        nc.scalar.dma_start(out=rows[:], in_=rows_ext[:])
        # contiguous 1024-col slices alternating the two spare rings; the
        # jc-outer loop consumes slice pairs 8 sigmoids at a time, so only
        # slices 0/1 gate the pipeline head
        for s in range(NSL):
            eng = nc.gpsimd if s % 2 == 0 else nc.scalar
            eng.dma_start(out=trep[:, s * SL:(s + 1) * SL], in_=trep_ext[s])
        nc.vector.memset(ones[:], 1.0)
        # warm the sigmoid table while input DMAs stream
        warm = singles.tile([P, 1], f32)
        nc.scalar.activation(out=warm[:], in_=ones[:], func=AF.Sigmoid)

        for jc in range(N_JC):
            j0 = jc * J_CHUNK
            for it in range(I_TILES):
                idx = it * N_JC + jc
                lh = lhst[:, it * P:(it + 1) * P]
                pd = psum.tile([P, J_CHUNK], f32, tag="pd")
                for h in range(J_CHUNK // MM_FREE):
                    a, b = h * MM_FREE, (h + 1) * MM_FREE
                    nc.tensor.matmul(pd[:, a:b], lh, rhsd[:, j0 + a:j0 + b])
                ra = ra_pool.tile([P, J_CHUNK], f32, tag="ra")
                nc.scalar.activation(
                    out=ra[:], in_=trep[:, j0:j0 + J_CHUNK], func=AF.Sigmoid,
                    scale=-float(BIGT), bias=tbias[:, it:it + 1],
                    accum_out=cacc[:, idx:idx + 1],
                )
                tr = tr_pool.tile([P, J_CHUNK], f32, tag="tr")
                nc.vector.scalar_tensor_tensor(
                    out=tr[:], in0=pd[:], scalar=0.0, in1=ra[:],
                    op0=OP.max, op1=OP.mult,
                    accum_out=hacc[:, idx:idx + 1],
                )

        # per-core MSE partial: sum((p_rows - t_rows)^2)
        nc.vector.tensor_tensor(out=diff[:], in0=prow[:], in1=trow[:], op=OP.subtract)
        nc.vector.scalar_tensor_tensor(
            out=diff2[:], in0=diff[:], scalar=0.0, in1=diff[:],
            op0=OP.add, op1=OP.mult, accum_out=stats[:, 2:3],
        )
        # fold 32 chunk accumulators
        nc.vector.tensor_reduce(stats[:, 0:1], hacc[:], axis=mybir.AxisListType.X, op=OP.add)
        nc.vector.tensor_reduce(stats[:, 1:2], cacc[:], axis=mybir.AxisListType.X, op=OP.add)
        # partition reduction: ones^T @ stats -> [1, 3]
        po = psum.tile([1, 3], f32, tag="pd")
        nc.tensor.matmul(po[:], ones[:], stats[:])
        nc.vector.tensor_copy(out=outs[:], in_=po[:])
        nc.sync.dma_start(out=out_ext[:], in_=outs[:])

    nc.finalize()
    return nc


class TileCtx:
    """TileContext + ExitStack bundle so _build reads linearly."""

    def __init__(self, nc):
        self.nc = nc

    def __enter__(self):
        self._stack = ExitStack()
        tc = self._stack.enter_context(tile.TileContext(self.nc))
        return tc, self._stack

    def __exit__(self, *exc):
        return self._stack.__exit__(*exc)


def _prep_inputs(p, t):
    p = np.ascontiguousarray(p, dtype=np.float32)
    t = np.ascontiguousarray(t, dtype=np.float32)
    ones = np.ones(N, dtype=ml_dtypes.bfloat16)
    pj0, pj1, pj2 = _split3(p)
    rhsd = np.zeros((KD, N), dtype=ml_dtypes.bfloat16)
    rhsd[0] = ones
    rhsd[1] = ones
    rhsd[2] = ones
    rhsd[3] = pj0
    rhsd[4] = pj1
    rhsd[5] = pj2
    NSL = 8
    SL = N // NSL
    # [8, 128, 1024]: slice-major replication, each slice contiguous
    trep = np.ascontiguousarray(
        np.broadcast_to(t.reshape(NSL, 1, SL), (NSL, P, SL)))
    in_maps = []
    ones_r = np.ones(ROWS_PER_CORE, dtype=ml_dtypes.bfloat16)
    for k in range(N_CORES):
        sl = slice(k * ROWS_PER_CORE, (k + 1) * ROWS_PER_CORE)
        pr, tr = p[sl], t[sl]
        s0, s1, s2 = _split3(MARGIN - pr)
        lhst = np.zeros((KD, ROWS_PER_CORE), dtype=ml_dtypes.bfloat16)
        lhst[0], lhst[1], lhst[2] = s0, s1, s2
        lhst[3], lhst[4], lhst[5] = ones_r, ones_r, ones_r
        mm6 = np.concatenate([lhst, rhsd], axis=1)
        MM_PIECE = (ROWS_PER_CORE + N) // 3
        mm6 = mm6.reshape(KD, 3, MM_PIECE).transpose(1, 0, 2)   # [3, KD, piece]
        rows = np.concatenate([
            pr.reshape(P, I_TILES, order="F"),
            tr.reshape(P, I_TILES, order="F"),
            (tr * BIGT).reshape(P, I_TILES, order="F"),
        ], axis=1)
        in_maps.append({
            "mm6": np.ascontiguousarray(mm6),
            "trep": trep,
            "rows": np.ascontiguousarray(rows.astype(np.float32)),
        })
    return p, t, in_maps


def _combine(p, t, results):
    hinge_p2 = sum(float(r["out"][0, 0]) for r in results)
    sig_cnt = sum(float(r["out"][0, 1]) for r in results)
    mse_sq = sum(float(r["out"][0, 2]) for r in results)

    # diagonal tie contribution: 0.5 * relu(d_ii), d_ii ~= margin
    dd = ((MARGIN - p) + p).astype(np.float32)
    diag = 0.5 * float(np.maximum(dd, np.float32(0.0)).sum(dtype=np.float64))

    hinge_sum = hinge_p2 - diag
    count = max(round(sig_cnt - 0.5 * N), 1.0)   # diagonal sigmoid(0) = 0.5
    mse = mse_sq / N
    loss = MSE_WEIGHT * mse + RANKING_WEIGHT * hinge_sum / count
    return np.float32(loss)


def _run(in_maps, trace=False):
    if "nc" not in _CACHE:
        _CACHE["nc"] = _build()
    return run_bass_kernel_spmd(
        _CACHE["nc"], in_maps, core_ids=list(range(N_CORES)), trace=trace,
    )


def kernel(predictions, targets):
    p, t, in_maps = _prep_inputs(predictions, targets)
    br = _run(in_maps)
    return _combine(p, t, br.results)
